# revision 14
# baseline (speedup 1.0000x reference)
"""GAT layer (gnn_message_passing) on 8 trn2 NeuronCores.

Strategy (dst-sharded, zero gathers, data-as-weights matmuls):
- Each core owns a contiguous 1/8 slice of target nodes; host buckets edges by
  dst core. Owned nodes are degree-sorted into 128-node windows; node -> SBUF
  partition, its in-edges occupy slot columns t=0..deg-1 (common T_w schedule
  across cores).
- Host lays out, per edge slot, the column [x[src](64) | edge_attr(16) |
  b_total(1)] into xe_slotT [81, SUMT*128] bf16 (pure indexed copy). Padded
  slots get -100 in row 80 so their logits vanish under exp.
- Device, per 128-slot block: ONE matmul with the slot data as the stationary
  operand: out[slot, :] = xe_blk.T @ WLG where WLG [81,132] packs
  [W_lin.T | fold(W_lin,w_s) + C(W_edge,w_e) + bias]. Column 0:128 = xp[src],
  128:132 = a_s[src]+a_e+b_total, already head-major (slots on partitions).
  a_t[dst] + residual come from one per-window matmul of xTown against
  [W_res.T+bias | fold(W_lin,w_t)].
- ACT copies psum->SBUF bf16; DVE adds a_t, leaky-relu (stt), ACT exp writes
  duplicated-pair ev straight into the msg tile; DVE multiplies ev into xp at
  bf16 2x rate (dup-pair broadcast AP keeps operands packed); per-window
  fold tree + axis-swapped tensor_reduce give numerator+denominator in one
  [128,136] result. out = num/denom + residual.
"""
import os
import sys
from contextlib import ExitStack

sys.path.insert(0, "/opt/trn_rl_repo")

import numpy as np
import ml_dtypes

BF16 = ml_dtypes.bfloat16

N, E = 50000, 1600000
IN_F, EDGE_F, HEADS, OUT_F = 64, 16, 4, 32
NEG_SLOPE = 0.2
NCORES = 8
NODES_PC = N // NCORES            # 6250
NW = (NODES_PC + 127) // 128      # 49 windows/core
WNODES = NW * 128                 # 6272
CH = 12                           # slot-cols per chunk (4 psum banks)
PAD_NEG = -100.0                  # row-80 value for invalid slots


def _host_preprocess(x, edge_index, edge_attr, W_lin, w_s, b_s, w_t, b_t,
                     W_edge, w_e, b_e, W_res, bias):
    """Pure index/layout work + weight folding. Returns (common, per_core)."""
    src = edge_index[0].astype(np.int64)
    dst = edge_index[1].astype(np.int64)
    deg = np.bincount(dst, minlength=N)

    # ---- weight folding (weights only; standard operator fusion) ----
    C = (W_edge.reshape(HEADS, OUT_F, EDGE_F) * w_e[None, :, None]).sum(1)  # [4,16]
    D = (W_lin.reshape(HEADS, OUT_F, IN_F) * w_t[None, :, None]).sum(1).T   # [64,4]
    Dws = (W_lin.reshape(HEADS, OUT_F, IN_F) * w_s[None, :, None]).sum(1).T  # [64,4]
    b_total = float(b_s) + float(b_t) + float(b_e)

    wlg = np.zeros((81, 132), np.float32)
    wlg[0:IN_F, 0:128] = W_lin.T.astype(np.float32)
    wlg[0:IN_F, 128:132] = Dws
    wlg[IN_F:80, 128:132] = C.T
    wlg[80, 128:132] = 1.0

    wrd = np.zeros((65, 132), np.float32)
    wrd[0:IN_F, 0:128] = W_res.T.astype(np.float32)
    wrd[IN_F, 0:128] = bias
    wrd[0:IN_F, 128:132] = D

    # ---- per-core schedules (common T_w across cores) ----
    cores = []
    for c in range(NCORES):
        lo = c * NODES_PC
        owned = np.arange(lo, lo + NODES_PC)
        dc = deg[owned]
        order = np.argsort(-dc, kind="stable")
        perm_owned = owned[order]
        dcs = dc[order]
        dcp = np.zeros(WNODES, np.int64)
        dcp[:NODES_PC] = dcs
        tw = dcp.reshape(NW, 128).max(axis=1)
        cores.append(dict(perm_owned=perm_owned, tw=np.maximum(tw, 1)))

    T_w = np.max(np.stack([cc["tw"] for cc in cores]), axis=0)  # [NW]
    TOFF = np.concatenate([[0], np.cumsum(T_w)])
    SUMT = int(TOFF[-1])

    per_core = []
    for c in range(NCORES):
        cc = cores[c]
        perm_owned = cc["perm_owned"]
        pos = np.empty(N, np.int64)
        pos[perm_owned] = np.arange(NODES_PC)

        emask = (dst >= c * NODES_PC) & (dst < (c + 1) * NODES_PC)
        e_ids = np.nonzero(emask)[0]
        d_loc = pos[dst[e_ids]]                      # 0..6249
        eorder = np.argsort(d_loc, kind="stable")
        e_s = e_ids[eorder]
        ds = d_loc[eorder]
        starts = np.searchsorted(ds, np.arange(NODES_PC))
        t_of = np.arange(len(ds)) - starts[ds]
        w_of = ds // 128
        p_of = ds % 128
        cols = (TOFF[w_of] + t_of) * 128 + p_of

        xe = np.zeros((SUMT * 128, 81), np.float32)
        xe[:, 80] = PAD_NEG
        xe[cols, 0:IN_F] = x[src[e_s]]
        xe[cols, IN_F:80] = edge_attr[e_s]
        xe[cols, 80] = b_total
        xeT = np.ascontiguousarray(xe.T).astype(BF16)

        xtown = np.zeros((65, WNODES), np.float32)
        xtown[0:IN_F, 0:NODES_PC] = x[perm_owned].T
        xtown[IN_F] = 1.0

        per_core.append(dict(
            xeT=xeT,
            xtown=xtown.astype(BF16),
            perm_owned=perm_owned,
        ))

    common = dict(T_w=T_w, TOFF=TOFF, SUMT=SUMT,
                  wlg=wlg.astype(BF16), wrd=wrd.astype(BF16))
    return common, per_core


def _build_program(common):
    import concourse.bass as bass
    import concourse.tile as tile
    from concourse import bacc, mybir

    f32 = mybir.dt.float32
    bf16 = mybir.dt.bfloat16
    AL = mybir.AluOpType
    AX = mybir.AxisListType
    T_w, TOFF, SUMT = common["T_w"], common["TOFF"], common["SUMT"]
    TMAX = int(T_w.max())

    nc = bacc.Bacc("TRN2", target_bir_lowering=False, debug=False,
                   num_devices=NCORES, num_swdge_queues=1)

    xe_d = nc.dram_tensor("xeT", [81, SUMT * 128], bf16, kind="ExternalInput")
    xt_d = nc.dram_tensor("xtown", [65, WNODES], bf16, kind="ExternalInput")
    wlg_d = nc.dram_tensor("wlg", [81, 132], bf16, kind="ExternalInput")
    wrd_d = nc.dram_tensor("wrd", [65, 132], bf16, kind="ExternalInput")
    out_d = nc.dram_tensor("out", [WNODES, 128], f32, kind="ExternalOutput")

    with tile.TileContext(nc) as tc, ExitStack() as ctx:
        const = ctx.enter_context(tc.tile_pool(name="const", bufs=1))
        wlg = const.tile([81, 132], bf16)
        nc.sync.dma_start(wlg[:], wlg_d.ap())
        wrd = const.tile([65, 132], bf16)
        nc.sync.dma_start(wrd[:], wrd_d.ap())
        xtown = const.tile([65, WNODES], bf16)
        nc.sync.dma_start(xtown[:], xt_d.ap())

        with tc.tile_pool(name="xep", bufs=4) as xep, \
             tc.tile_pool(name="xsp", bufs=3) as xsp, \
             tc.tile_pool(name="msgp", bufs=2) as msgp, \
             tc.tile_pool(name="up", bufs=3) as up, \
             tc.tile_pool(name="resatp", bufs=2) as resatp, \
             tc.tile_pool(name="foldp", bufs=2) as foldp, \
             tc.tile_pool(name="outp", bufs=3) as outp, \
             tc.tile_pool(name="psp", bufs=2, space="PSUM") as psp:

            pend = [None]          # (t0, tn, xs, close_after, emit_mult)
            def flush_pend():
                if pend[0] is None:
                    return
                t0, tn, xs, close_after, em = pend[0]
                pend[0] = None
                em(t0, tn, xs)
                if close_after is not None:
                    close_after()

            for w in range(NW):
                T = int(T_w[w])
                # residual + a_t for this window's 128 dst nodes
                ps_r = psp.tile([128, 4, 512], f32, tag="blk")
                nc.tensor.matmul(ps_r[:, 0, 0:132],
                                 xtown[:, w * 128:(w + 1) * 128], wrd[:],
                                 start=True, stop=True)
                resat = resatp.tile([128, 132], bf16, tag="resat")
                nc.scalar.copy(resat[:], ps_r[:, 0, 0:132])

                msg = msgp.tile([128, TMAX, 128], bf16, tag="msg")
                evd = msgp.tile([128, TMAX, 4, 2], bf16, tag="evd")

                def emit_mult(t0, tn, xs, msg=msg, evd=evd):
                    # msg = ev * xp  (bf16 2x: dup-pair broadcast keeps packed;
                    # (t,h) dims of the ev operand merge -> 3 free dims)
                    evb = evd[:, t0:t0 + tn] \
                        .rearrange("p t h (a two) -> p t h a two", a=1, two=2) \
                        .broadcast_to([128, tn, 4, 16, 2])
                    nc.vector.tensor_tensor(
                        msg[:, t0:t0 + tn, :]
                            .rearrange("p t (h a two) -> p t h a two", h=4, two=2),
                        xs[:, :tn, 0:128]
                            .rearrange("p t (h a two) -> p t h a two", h=4, two=2),
                        evb, op=AL.mult)

                def emit_close(w=w, T=T, msg=msg, evd=evd, resat=resat):
                    # ---- window fold: numerator tree + denominator reduce ----
                    n = T
                    while n > 2:
                        k = n // 2
                        nc.vector.tensor_tensor(
                            msg[:, 0:k, :], msg[:, 0:k, :], msg[:, n - k:n, :],
                            op=AL.add)
                        n -= k
                    fold = foldp.tile([128, 128], f32, tag="fold")
                    if T >= 2:
                        nc.gpsimd.tensor_tensor(fold[:], msg[:, 0, :],
                                                msg[:, 1, :], op=AL.add)
                    else:
                        nc.gpsimd.tensor_copy(fold[:], msg[:, 0, :])
                    den8 = foldp.tile([128, 8], f32, tag="den")
                    nc.vector.tensor_reduce(
                        den8[:],
                        evd[:, 0:T].rearrange("p t h two -> p (h two) t"),
                        axis=AX.X, op=AL.add)
                    # ---- close: out = num/denom + residual ----
                    rec8 = foldp.tile([128, 8], f32, tag="rec")
                    nc.vector.reciprocal(rec8[:], den8[:])
                    outw = outp.tile([128, 128], f32, tag="outw")
                    recb = rec8[:] \
                        .rearrange("p (h a two) -> p h a two", a=1, two=2) \
                        .broadcast_to([128, 4, 16, 2])
                    nc.vector.tensor_tensor(
                        outw[:].rearrange("p (h a two) -> p h a two",
                                          h=4, two=2),
                        fold[:].rearrange("p (h a two) -> p h a two",
                                          h=4, two=2),
                        recb, op=AL.mult)
                    out2 = outp.tile([128, 128], f32, tag="out2")
                    nc.gpsimd.tensor_tensor(out2[:], outw[:], resat[:, 0:128],
                                            op=AL.add)
                    nc.sync.dma_start(out_d.ap()[w * 128:(w + 1) * 128, :],
                                      out2[:])

                t0 = 0
                while t0 < T:
                    tn = min(CH, T - t0)
                    gc0 = (int(TOFF[w]) + t0) * 128
                    xe = xep.tile([81, CH, 128], bf16, tag="xe")
                    nc.sync.dma_start(
                        xe[:, :tn, :],
                        xe_d.ap()[:, gc0: gc0 + tn * 128]
                            .rearrange("p (t c) -> p t c", c=128))
                    ps = psp.tile([128, 4, 512], f32, tag="blk")
                    for j in range(tn):
                        nc.tensor.matmul(
                            ps[:, j // 3, (j % 3) * 132:(j % 3) * 132 + 132],
                            xe[:, j, :], wlg[:], start=True, stop=True)
                    psv = ps[:, :, 0:396].rearrange("p b (j c) -> p b j c", c=132)
                    # evacuate xp + u_pre to SBUF bf16 (ACT)
                    xs = xsp.tile([128, CH, 132], bf16, tag="xs")
                    nc.scalar.copy(
                        xs[:].rearrange("p (b j) c -> p b j c", b=4), psv)
                    # software-pipeline: big multiply + window close run late
                    flush_pend()
                    # logits: u = (a_s + a_e + b_total) + a_t ; leaky-relu; exp
                    u = up.tile([128, CH, 4], bf16, tag="u")
                    atb = resat[:, 128:132] \
                        .rearrange("p (a h) -> p a h", a=1) \
                        .broadcast_to([128, CH, 4])
                    nc.vector.tensor_tensor(u[:], xs[:, :, 128:132], atb,
                                            op=AL.add)
                    lr = up.tile([128, CH, 4], bf16, tag="lr")
                    nc.vector.scalar_tensor_tensor(lr[:], u[:], NEG_SLOPE, u[:],
                                                   op0=AL.mult, op1=AL.max)
                    # exp -> duplicated-pair ev
                    lrb = lr[:, :tn, :].rearrange("p t (h a) -> p t h a", a=1) \
                        .broadcast_to([128, tn, 4, 2])
                    nc.scalar.activation(evd[:, t0:t0 + tn], lrb,
                                         mybir.ActivationFunctionType.Exp)
                    t0 += tn
                    is_last = t0 >= T
                    pend[0] = (t0 - tn, tn, xs,
                               emit_close if is_last else None, emit_mult)
            flush_pend()

    nc.compile()
    return nc


def kernel(**inputs):
    from concourse.bass_utils import run_bass_kernel_spmd

    args = {k: np.asarray(v) for k, v in inputs.items()}
    common, per_core = _host_preprocess(
        args["x"], args["edge_index"], args["edge_attr"], args["W_lin"],
        args["w_s"], args["b_s"], args["w_t"], args["b_t"], args["W_edge"],
        args["w_e"], args["b_e"], args["W_res"], args["bias"])

    nc = _build_program(common)

    in_maps = []
    for c in range(NCORES):
        pc = per_core[c]
        in_maps.append({
            "xeT": pc["xeT"], "xtown": pc["xtown"],
            "wlg": common["wlg"], "wrd": common["wrd"],
        })

    res = run_bass_kernel_spmd(nc, in_maps, list(range(NCORES)),
                               trace=bool(os.environ.get("GAT_TRACE")),
                               tmpdir=os.environ.get("GAT_TMPDIR"))
    if os.environ.get("GAT_TRACE"):
        print(f"HW exec time: {res.exec_time_ns} ns")

    out = np.empty((N, HEADS * OUT_F), np.float32)
    for c in range(NCORES):
        out[per_core[c]["perm_owned"]] = res.results[c]["out"][:NODES_PC]
    return out


# revision 15
# speedup vs baseline: 1.0568x; 1.0568x over previous
"""GAT layer (gnn_message_passing) on 8 trn2 NeuronCores.

Strategy (dst-sharded, zero gathers, data-as-weights matmuls):
- Each core owns a contiguous 1/8 slice of target nodes; host buckets edges by
  dst core. Owned nodes are degree-sorted into 128-node windows; node -> SBUF
  partition, its in-edges occupy slot columns t=0..deg-1 (common T_w schedule
  across cores).
- Host lays out, per edge slot, the column [x[src](64) | edge_attr(16) |
  b_total(1)] into xe_slotT [81, SUMT*128] bf16 (pure indexed copy). Padded
  slots get -100 in row 80 so their logits vanish under exp.
- Device, per 128-slot block: ONE matmul with the slot data as the stationary
  operand: out[slot, :] = xe_blk.T @ WLG where WLG [81,132] packs
  [W_lin.T | fold(W_lin,w_s) + C(W_edge,w_e) + bias]. Column 0:128 = xp[src],
  128:132 = a_s[src]+a_e+b_total, already head-major (slots on partitions).
  a_t[dst] + residual come from one per-window matmul of xTown against
  [W_res.T+bias | fold(W_lin,w_t)].
- ACT copies psum->SBUF bf16; DVE adds a_t, leaky-relu (stt), ACT exp writes
  duplicated-pair ev straight into the msg tile; DVE multiplies ev into xp at
  bf16 2x rate (dup-pair broadcast AP keeps operands packed); per-window
  fold tree + axis-swapped tensor_reduce give numerator+denominator in one
  [128,136] result. out = num/denom + residual.
"""
import os
import sys
from contextlib import ExitStack

sys.path.insert(0, "/opt/trn_rl_repo")

import numpy as np
import ml_dtypes

BF16 = ml_dtypes.bfloat16

N, E = 50000, 1600000
IN_F, EDGE_F, HEADS, OUT_F = 64, 16, 4, 32
NEG_SLOPE = 0.2
NCORES = 8
NODES_PC = N // NCORES            # 6250
NW = (NODES_PC + 127) // 128      # 49 windows/core
WNODES = NW * 128                 # 6272
CH = 12                           # slot-cols per chunk (4 psum banks)
PAD_NEG = -100.0                  # row-80 value for invalid slots


def _host_preprocess(x, edge_index, edge_attr, W_lin, w_s, b_s, w_t, b_t,
                     W_edge, w_e, b_e, W_res, bias):
    """Pure index/layout work + weight folding. Returns (common, per_core)."""
    src = edge_index[0].astype(np.int64)
    dst = edge_index[1].astype(np.int64)
    deg = np.bincount(dst, minlength=N)

    # ---- weight folding (weights only; standard operator fusion) ----
    C = (W_edge.reshape(HEADS, OUT_F, EDGE_F) * w_e[None, :, None]).sum(1)  # [4,16]
    D = (W_lin.reshape(HEADS, OUT_F, IN_F) * w_t[None, :, None]).sum(1).T   # [64,4]
    Dws = (W_lin.reshape(HEADS, OUT_F, IN_F) * w_s[None, :, None]).sum(1).T  # [64,4]
    b_total = float(b_s) + float(b_t) + float(b_e)

    wlg = np.zeros((81, 132), np.float32)
    wlg[0:IN_F, 0:128] = W_lin.T.astype(np.float32)
    wlg[0:IN_F, 128:132] = Dws
    wlg[IN_F:80, 128:132] = C.T
    wlg[80, 128:132] = 1.0

    wrd = np.zeros((65, 132), np.float32)
    wrd[0:IN_F, 0:128] = W_res.T.astype(np.float32)
    wrd[IN_F, 0:128] = bias
    wrd[0:IN_F, 128:132] = D

    # ---- per-core schedules (common T_w across cores) ----
    cores = []
    for c in range(NCORES):
        lo = c * NODES_PC
        owned = np.arange(lo, lo + NODES_PC)
        dc = deg[owned]
        order = np.argsort(-dc, kind="stable")
        perm_owned = owned[order]
        dcs = dc[order]
        dcp = np.zeros(WNODES, np.int64)
        dcp[:NODES_PC] = dcs
        tw = dcp.reshape(NW, 128).max(axis=1)
        cores.append(dict(perm_owned=perm_owned, tw=np.maximum(tw, 1)))

    T_w = np.max(np.stack([cc["tw"] for cc in cores]), axis=0)  # [NW]
    TOFF = np.concatenate([[0], np.cumsum(T_w)])
    SUMT = int(TOFF[-1])

    per_core = []
    for c in range(NCORES):
        cc = cores[c]
        perm_owned = cc["perm_owned"]
        pos = np.empty(N, np.int64)
        pos[perm_owned] = np.arange(NODES_PC)

        emask = (dst >= c * NODES_PC) & (dst < (c + 1) * NODES_PC)
        e_ids = np.nonzero(emask)[0]
        d_loc = pos[dst[e_ids]]                      # 0..6249
        eorder = np.argsort(d_loc, kind="stable")
        e_s = e_ids[eorder]
        ds = d_loc[eorder]
        starts = np.searchsorted(ds, np.arange(NODES_PC))
        t_of = np.arange(len(ds)) - starts[ds]
        w_of = ds // 128
        p_of = ds % 128
        cols = (TOFF[w_of] + t_of) * 128 + p_of

        xe = np.zeros((SUMT * 128, 81), np.float32)
        xe[:, 80] = PAD_NEG
        xe[cols, 0:IN_F] = x[src[e_s]]
        xe[cols, IN_F:80] = edge_attr[e_s]
        xe[cols, 80] = b_total
        xeT = np.ascontiguousarray(xe.T).astype(BF16)

        xtown = np.zeros((65, WNODES), np.float32)
        xtown[0:IN_F, 0:NODES_PC] = x[perm_owned].T
        xtown[IN_F] = 1.0

        per_core.append(dict(
            xeT=xeT,
            xtown=xtown.astype(BF16),
            perm_owned=perm_owned,
        ))

    common = dict(T_w=T_w, TOFF=TOFF, SUMT=SUMT,
                  wlg=wlg.astype(BF16), wrd=wrd.astype(BF16))
    return common, per_core


def _build_program(common):
    import concourse.bass as bass
    import concourse.tile as tile
    from concourse import bacc, mybir

    f32 = mybir.dt.float32
    bf16 = mybir.dt.bfloat16
    AL = mybir.AluOpType
    AX = mybir.AxisListType
    T_w, TOFF, SUMT = common["T_w"], common["TOFF"], common["SUMT"]
    TMAX = int(T_w.max())

    nc = bacc.Bacc("TRN2", target_bir_lowering=False, debug=False,
                   num_devices=NCORES, num_swdge_queues=1)

    xe_d = nc.dram_tensor("xeT", [81, SUMT * 128], bf16, kind="ExternalInput")
    xt_d = nc.dram_tensor("xtown", [65, WNODES], bf16, kind="ExternalInput")
    wlg_d = nc.dram_tensor("wlg", [81, 132], bf16, kind="ExternalInput")
    wrd_d = nc.dram_tensor("wrd", [65, 132], bf16, kind="ExternalInput")
    out_d = nc.dram_tensor("out", [WNODES, 128], f32, kind="ExternalOutput")

    with tile.TileContext(nc) as tc, ExitStack() as ctx:
        const = ctx.enter_context(tc.tile_pool(name="const", bufs=1))
        wlg = const.tile([81, 132], bf16)
        nc.sync.dma_start(wlg[:], wlg_d.ap())
        wrd = const.tile([65, 132], bf16)
        nc.sync.dma_start(wrd[:], wrd_d.ap())
        xtown = const.tile([65, WNODES], bf16)
        nc.sync.dma_start(xtown[:], xt_d.ap())

        with tc.tile_pool(name="xep", bufs=8) as xep, \
             tc.tile_pool(name="xsp", bufs=5) as xsp, \
             tc.tile_pool(name="msgp", bufs=2) as msgp, \
             tc.tile_pool(name="up", bufs=6) as up, \
             tc.tile_pool(name="resatp", bufs=3) as resatp, \
             tc.tile_pool(name="foldp", bufs=4) as foldp, \
             tc.tile_pool(name="outp", bufs=4) as outp, \
             tc.tile_pool(name="psp", bufs=2, space="PSUM") as psp:

            pend = [None]          # (t0, tn, xs, close_after, emit_mult)
            def flush_pend():
                if pend[0] is None:
                    return
                t0, tn, xs, close_after, em = pend[0]
                pend[0] = None
                em(t0, tn, xs)
                if close_after is not None:
                    close_after()

            for w in range(NW):
                T = int(T_w[w])
                # residual + a_t for this window's 128 dst nodes
                ps_r = psp.tile([128, 4, 512], f32, tag="blk")
                nc.tensor.matmul(ps_r[:, 0, 0:132],
                                 xtown[:, w * 128:(w + 1) * 128], wrd[:],
                                 start=True, stop=True)
                resat = resatp.tile([128, 132], bf16, tag="resat")
                nc.scalar.copy(resat[:], ps_r[:, 0, 0:132])

                msg = msgp.tile([128, TMAX, 128], bf16, tag="msg")
                evd = msgp.tile([128, TMAX, 4, 2], bf16, tag="evd")

                def emit_mult(t0, tn, xs, msg=msg, evd=evd):
                    # msg = ev * xp  (bf16 2x: dup-pair broadcast keeps packed;
                    # (t,h) dims of the ev operand merge -> 3 free dims)
                    evb = evd[:, t0:t0 + tn] \
                        .rearrange("p t h (a two) -> p t h a two", a=1, two=2) \
                        .broadcast_to([128, tn, 4, 16, 2])
                    nc.vector.tensor_tensor(
                        msg[:, t0:t0 + tn, :]
                            .rearrange("p t (h a two) -> p t h a two", h=4, two=2),
                        xs[:, :tn, 0:128]
                            .rearrange("p t (h a two) -> p t h a two", h=4, two=2),
                        evb, op=AL.mult)

                def emit_close(w=w, T=T, msg=msg, evd=evd, resat=resat):
                    # ---- window fold: numerator tree + denominator reduce ----
                    n = T
                    while n > 2:
                        k = n // 2
                        nc.vector.tensor_tensor(
                            msg[:, 0:k, :], msg[:, 0:k, :], msg[:, n - k:n, :],
                            op=AL.add)
                        n -= k
                    fold = foldp.tile([128, 128], f32, tag="fold")
                    if T >= 2:
                        nc.gpsimd.tensor_tensor(fold[:], msg[:, 0, :],
                                                msg[:, 1, :], op=AL.add)
                    else:
                        nc.gpsimd.tensor_copy(fold[:], msg[:, 0, :])
                    den8 = foldp.tile([128, 8], f32, tag="den")
                    nc.vector.tensor_reduce(
                        den8[:],
                        evd[:, 0:T].rearrange("p t h two -> p (h two) t"),
                        axis=AX.X, op=AL.add)
                    # ---- close: out = num/denom + residual ----
                    rec8 = foldp.tile([128, 8], f32, tag="rec")
                    nc.vector.reciprocal(rec8[:], den8[:])
                    outw = outp.tile([128, 128], f32, tag="outw")
                    recb = rec8[:] \
                        .rearrange("p (h a two) -> p h a two", a=1, two=2) \
                        .broadcast_to([128, 4, 16, 2])
                    nc.vector.tensor_tensor(
                        outw[:].rearrange("p (h a two) -> p h a two",
                                          h=4, two=2),
                        fold[:].rearrange("p (h a two) -> p h a two",
                                          h=4, two=2),
                        recb, op=AL.mult)
                    out2 = outp.tile([128, 128], f32, tag="out2")
                    nc.gpsimd.tensor_tensor(out2[:], outw[:], resat[:, 0:128],
                                            op=AL.add)
                    nc.sync.dma_start(out_d.ap()[w * 128:(w + 1) * 128, :],
                                      out2[:])

                t0 = 0
                while t0 < T:
                    tn = min(CH, T - t0)
                    gc0 = (int(TOFF[w]) + t0) * 128
                    xe = xep.tile([81, CH, 128], bf16, tag="xe")
                    nc.sync.dma_start(
                        xe[:, :tn, :],
                        xe_d.ap()[:, gc0: gc0 + tn * 128]
                            .rearrange("p (t c) -> p t c", c=128))
                    ps = psp.tile([128, 4, 512], f32, tag="blk")
                    for j in range(tn):
                        nc.tensor.matmul(
                            ps[:, j // 3, (j % 3) * 132:(j % 3) * 132 + 132],
                            xe[:, j, :], wlg[:], start=True, stop=True)
                    psv = ps[:, :, 0:396].rearrange("p b (j c) -> p b j c", c=132)
                    # evacuate xp + u_pre to SBUF bf16 (ACT)
                    xs = xsp.tile([128, CH, 132], bf16, tag="xs")
                    nc.scalar.copy(
                        xs[:].rearrange("p (b j) c -> p b j c", b=4), psv)
                    # software-pipeline: big multiply + window close run late
                    flush_pend()
                    # logits: u = (a_s + a_e + b_total) + a_t ; leaky-relu; exp
                    u = up.tile([128, CH, 4], bf16, tag="u")
                    atb = resat[:, 128:132] \
                        .rearrange("p (a h) -> p a h", a=1) \
                        .broadcast_to([128, CH, 4])
                    nc.vector.tensor_tensor(u[:], xs[:, :, 128:132], atb,
                                            op=AL.add)
                    lr = up.tile([128, CH, 4], bf16, tag="lr")
                    nc.vector.scalar_tensor_tensor(lr[:], u[:], NEG_SLOPE, u[:],
                                                   op0=AL.mult, op1=AL.max)
                    # exp -> duplicated-pair ev
                    lrb = lr[:, :tn, :].rearrange("p t (h a) -> p t h a", a=1) \
                        .broadcast_to([128, tn, 4, 2])
                    nc.scalar.activation(evd[:, t0:t0 + tn], lrb,
                                         mybir.ActivationFunctionType.Exp)
                    t0 += tn
                    is_last = t0 >= T
                    pend[0] = (t0 - tn, tn, xs,
                               emit_close if is_last else None, emit_mult)
            flush_pend()

    nc.compile()
    return nc


def kernel(**inputs):
    from concourse.bass_utils import run_bass_kernel_spmd

    args = {k: np.asarray(v) for k, v in inputs.items()}
    common, per_core = _host_preprocess(
        args["x"], args["edge_index"], args["edge_attr"], args["W_lin"],
        args["w_s"], args["b_s"], args["w_t"], args["b_t"], args["W_edge"],
        args["w_e"], args["b_e"], args["W_res"], args["bias"])

    nc = _build_program(common)

    in_maps = []
    for c in range(NCORES):
        pc = per_core[c]
        in_maps.append({
            "xeT": pc["xeT"], "xtown": pc["xtown"],
            "wlg": common["wlg"], "wrd": common["wrd"],
        })

    res = run_bass_kernel_spmd(nc, in_maps, list(range(NCORES)),
                               trace=bool(os.environ.get("GAT_TRACE")),
                               tmpdir=os.environ.get("GAT_TMPDIR"))
    if os.environ.get("GAT_TRACE"):
        print(f"HW exec time: {res.exec_time_ns} ns")

    out = np.empty((N, HEADS * OUT_F), np.float32)
    for c in range(NCORES):
        out[per_core[c]["perm_owned"]] = res.results[c]["out"][:NODES_PC]
    return out


# revision 17
# speedup vs baseline: 1.0813x; 1.0231x over previous
"""GAT layer (gnn_message_passing) on 8 trn2 NeuronCores.

Strategy (dst-sharded, zero gathers, data-as-weights matmuls):
- Each core owns a contiguous 1/8 slice of target nodes; host buckets edges by
  dst core. Owned nodes are degree-sorted into 128-node windows; node -> SBUF
  partition, its in-edges occupy slot columns t=0..deg-1 (common T_w schedule
  across cores).
- Host lays out, per edge slot, the column [x[src](64) | edge_attr(16) |
  b_total(1)] into xe_slotT [81, SUMT*128] bf16 (pure indexed copy). Padded
  slots get -100 in row 80 so their logits vanish under exp.
- Device, per 128-slot block: ONE matmul with the slot data as the stationary
  operand: out[slot, :] = xe_blk.T @ WLG where WLG [81,132] packs
  [W_lin.T | fold(W_lin,w_s) + C(W_edge,w_e) + bias]. Column 0:128 = xp[src],
  128:132 = a_s[src]+a_e+b_total, already head-major (slots on partitions).
  a_t[dst] + residual come from one per-window matmul of xTown against
  [W_res.T+bias | fold(W_lin,w_t)].
- ACT copies psum->SBUF bf16; DVE adds a_t, leaky-relu (stt), ACT exp writes
  duplicated-pair ev straight into the msg tile; DVE multiplies ev into xp at
  bf16 2x rate (dup-pair broadcast AP keeps operands packed); per-window
  fold tree + axis-swapped tensor_reduce give numerator+denominator in one
  [128,136] result. out = num/denom + residual.
"""
import os
import sys
from contextlib import ExitStack

sys.path.insert(0, "/opt/trn_rl_repo")

import numpy as np
import ml_dtypes

BF16 = ml_dtypes.bfloat16

N, E = 50000, 1600000
IN_F, EDGE_F, HEADS, OUT_F = 64, 16, 4, 32
NEG_SLOPE = 0.2
NCORES = 8
NODES_PC = N // NCORES            # 6250
NW = (NODES_PC + 127) // 128      # 49 windows/core
WNODES = NW * 128                 # 6272
CH = 12                           # slot-cols per chunk (4 psum banks)
PAD_NEG = -100.0                  # row-80 value for invalid slots


def _host_preprocess(x, edge_index, edge_attr, W_lin, w_s, b_s, w_t, b_t,
                     W_edge, w_e, b_e, W_res, bias):
    """Pure index/layout work + weight folding. Returns (common, per_core)."""
    src = edge_index[0].astype(np.int64)
    dst = edge_index[1].astype(np.int64)
    deg = np.bincount(dst, minlength=N)

    # ---- weight folding (weights only; standard operator fusion) ----
    C = (W_edge.reshape(HEADS, OUT_F, EDGE_F) * w_e[None, :, None]).sum(1)  # [4,16]
    D = (W_lin.reshape(HEADS, OUT_F, IN_F) * w_t[None, :, None]).sum(1).T   # [64,4]
    Dws = (W_lin.reshape(HEADS, OUT_F, IN_F) * w_s[None, :, None]).sum(1).T  # [64,4]
    b_total = float(b_s) + float(b_t) + float(b_e)

    wlg = np.zeros((81, 132), np.float32)
    wlg[0:IN_F, 0:128] = W_lin.T.astype(np.float32)
    wlg[0:IN_F, 128:132] = Dws
    wlg[IN_F:80, 128:132] = C.T
    wlg[80, 128:132] = 1.0

    wrd = np.zeros((65, 132), np.float32)
    wrd[0:IN_F, 0:128] = W_res.T.astype(np.float32)
    wrd[IN_F, 0:128] = bias
    wrd[0:IN_F, 128:132] = D

    # ---- per-core schedules (common T_w across cores) ----
    cores = []
    for c in range(NCORES):
        lo = c * NODES_PC
        owned = np.arange(lo, lo + NODES_PC)
        dc = deg[owned]
        order = np.argsort(-dc, kind="stable")
        perm_owned = owned[order]
        dcs = dc[order]
        dcp = np.zeros(WNODES, np.int64)
        dcp[:NODES_PC] = dcs
        tw = dcp.reshape(NW, 128).max(axis=1)
        cores.append(dict(perm_owned=perm_owned, tw=np.maximum(tw, 1)))

    T_w = np.max(np.stack([cc["tw"] for cc in cores]), axis=0)  # [NW]
    TOFF = np.concatenate([[0], np.cumsum(T_w)])
    SUMT = int(TOFF[-1])

    per_core = []
    for c in range(NCORES):
        cc = cores[c]
        perm_owned = cc["perm_owned"]
        pos = np.empty(N, np.int64)
        pos[perm_owned] = np.arange(NODES_PC)

        emask = (dst >= c * NODES_PC) & (dst < (c + 1) * NODES_PC)
        e_ids = np.nonzero(emask)[0]
        d_loc = pos[dst[e_ids]]                      # 0..6249
        eorder = np.argsort(d_loc, kind="stable")
        e_s = e_ids[eorder]
        ds = d_loc[eorder]
        starts = np.searchsorted(ds, np.arange(NODES_PC))
        t_of = np.arange(len(ds)) - starts[ds]
        w_of = ds // 128
        p_of = ds % 128
        cols = (TOFF[w_of] + t_of) * 128 + p_of

        xe = np.zeros((SUMT * 128, 81), np.float32)
        xe[:, 80] = PAD_NEG
        xe[cols, 0:IN_F] = x[src[e_s]]
        xe[cols, IN_F:80] = edge_attr[e_s]
        xe[cols, 80] = b_total
        xeT = np.ascontiguousarray(xe.T).astype(BF16)

        xtown = np.zeros((65, WNODES), np.float32)
        xtown[0:IN_F, 0:NODES_PC] = x[perm_owned].T
        xtown[IN_F] = 1.0

        per_core.append(dict(
            xeT=xeT,
            xtown=xtown.astype(BF16),
            perm_owned=perm_owned,
        ))

    common = dict(T_w=T_w, TOFF=TOFF, SUMT=SUMT,
                  wlg=wlg.astype(BF16), wrd=wrd.astype(BF16))
    return common, per_core


def _build_program(common):
    import concourse.bass as bass
    import concourse.tile as tile
    from concourse import bacc, mybir

    f32 = mybir.dt.float32
    bf16 = mybir.dt.bfloat16
    AL = mybir.AluOpType
    AX = mybir.AxisListType
    T_w, TOFF, SUMT = common["T_w"], common["TOFF"], common["SUMT"]
    TMAX = int(T_w.max())

    nc = bacc.Bacc("TRN2", target_bir_lowering=False, debug=False,
                   num_devices=NCORES, num_swdge_queues=1)

    xe_d = nc.dram_tensor("xeT", [81, SUMT * 128], bf16, kind="ExternalInput")
    xt_d = nc.dram_tensor("xtown", [65, WNODES], bf16, kind="ExternalInput")
    wlg_d = nc.dram_tensor("wlg", [81, 132], bf16, kind="ExternalInput")
    wrd_d = nc.dram_tensor("wrd", [65, 132], bf16, kind="ExternalInput")
    out_d = nc.dram_tensor("out", [WNODES, 128], f32, kind="ExternalOutput")

    with tile.TileContext(nc) as tc, ExitStack() as ctx:
        const = ctx.enter_context(tc.tile_pool(name="const", bufs=1))
        wlg = const.tile([81, 132], bf16)
        nc.sync.dma_start(wlg[:], wlg_d.ap())
        wrd = const.tile([65, 132], bf16)
        nc.sync.dma_start(wrd[:], wrd_d.ap())
        xtown = const.tile([65, WNODES], bf16)
        nc.sync.dma_start(xtown[:], xt_d.ap())

        with tc.tile_pool(name="xep", bufs=8) as xep, \
             tc.tile_pool(name="xsp", bufs=3) as xsp, \
             tc.tile_pool(name="msgp", bufs=2) as msgp, \
             tc.tile_pool(name="up", bufs=3) as up, \
             tc.tile_pool(name="resatp", bufs=3) as resatp, \
             tc.tile_pool(name="foldp", bufs=4) as foldp, \
             tc.tile_pool(name="outp", bufs=4) as outp, \
             tc.tile_pool(name="psp", bufs=2, space="PSUM") as psp:

            pend = [None]      # (t0, bw, xs, close_after, emit_mult)
            def flush_pend():
                if pend[0] is None:
                    return
                t0, bw, xs, close_after, em = pend[0]
                pend[0] = None
                em(t0, bw, xs)
                if close_after is not None:
                    close_after()

            BUN = 4                # chunks per logit/mult bundle
            for w in range(NW):
                T = int(T_w[w])
                # residual + a_t for this window's 128 dst nodes
                ps_r = psp.tile([128, 4, 512], f32, tag="blk")
                nc.tensor.matmul(ps_r[:, 0, 0:132],
                                 xtown[:, w * 128:(w + 1) * 128], wrd[:],
                                 start=True, stop=True)
                resat = resatp.tile([128, 132], bf16, tag="resat")
                nc.scalar.copy(resat[:], ps_r[:, 0, 0:132])

                msg = msgp.tile([128, TMAX, 128], bf16, tag="msg")
                evd = msgp.tile([128, TMAX, 4, 2], bf16, tag="evd")

                def emit_mult(t0, bw, xs, msg=msg, evd=evd):
                    # msg = ev * xp  (bf16 2x: dup-pair broadcast keeps packed;
                    # (t,h) dims of the ev operand merge -> 3 free dims)
                    evb = evd[:, t0:t0 + bw] \
                        .rearrange("p t h (a two) -> p t h a two", a=1, two=2) \
                        .broadcast_to([128, bw, 4, 16, 2])
                    nc.vector.tensor_tensor(
                        msg[:, t0:t0 + bw, :]
                            .rearrange("p t (h a two) -> p t h a two", h=4, two=2),
                        xs[:, :bw, 0:128]
                            .rearrange("p t (h a two) -> p t h a two", h=4, two=2),
                        evb, op=AL.mult)

                def emit_close(w=w, T=T, msg=msg, evd=evd, resat=resat):
                    # ---- window fold: numerator tree + denominator reduce ----
                    n = T
                    while n > 2:
                        k = n // 2
                        nc.vector.tensor_tensor(
                            msg[:, 0:k, :], msg[:, 0:k, :], msg[:, n - k:n, :],
                            op=AL.add)
                        n -= k
                    fold = foldp.tile([128, 128], f32, tag="fold")
                    if T >= 2:
                        nc.gpsimd.tensor_tensor(fold[:], msg[:, 0, :],
                                                msg[:, 1, :], op=AL.add)
                    else:
                        nc.gpsimd.tensor_copy(fold[:], msg[:, 0, :])
                    den8 = foldp.tile([128, 8], f32, tag="den")
                    nc.vector.tensor_reduce(
                        den8[:],
                        evd[:, 0:T].rearrange("p t h two -> p (h two) t"),
                        axis=AX.X, op=AL.add)
                    # ---- close: out = num/denom + residual ----
                    rec8 = foldp.tile([128, 8], f32, tag="rec")
                    nc.vector.reciprocal(rec8[:], den8[:])
                    outw = outp.tile([128, 128], f32, tag="outw")
                    recb = rec8[:] \
                        .rearrange("p (h a two) -> p h a two", a=1, two=2) \
                        .broadcast_to([128, 4, 16, 2])
                    nc.vector.tensor_tensor(
                        outw[:].rearrange("p (h a two) -> p h a two",
                                          h=4, two=2),
                        fold[:].rearrange("p (h a two) -> p h a two",
                                          h=4, two=2),
                        recb, op=AL.mult)
                    out2 = outp.tile([128, 128], f32, tag="out2")
                    nc.gpsimd.tensor_tensor(out2[:], outw[:], resat[:, 0:128],
                                            op=AL.add)
                    nc.sync.dma_start(out_d.ap()[w * 128:(w + 1) * 128, :],
                                      out2[:])

                def emit_logits(bt0, bw, xs, resat=resat, evd=evd):
                    # u = (a_s + a_e + b_total) + a_t ; leaky-relu; exp
                    u = up.tile([128, BUN * CH, 4], bf16, tag="u")
                    atb = resat[:, 128:132] \
                        .rearrange("p (a h) -> p a h", a=1) \
                        .broadcast_to([128, bw, 4])
                    nc.vector.tensor_tensor(u[:, :bw], xs[:, :bw, 128:132],
                                            atb, op=AL.add)
                    lr = up.tile([128, BUN * CH, 4], bf16, tag="lr")
                    nc.vector.scalar_tensor_tensor(lr[:, :bw], u[:, :bw],
                                                   NEG_SLOPE, u[:, :bw],
                                                   op0=AL.mult, op1=AL.max)
                    lrb = lr[:, :bw, :].rearrange("p t (h a) -> p t h a", a=1) \
                        .broadcast_to([128, bw, 4, 2])
                    nc.scalar.activation(evd[:, bt0:bt0 + bw], lrb,
                                         mybir.ActivationFunctionType.Exp)

                nch = (T + CH - 1) // CH
                ci = 0
                t0 = 0
                while t0 < T:
                    # bundle of up to BUN chunks sharing one xs tile
                    nbun = min(BUN, nch - ci)
                    bt0 = t0
                    xs = xsp.tile([128, BUN * CH, 132], bf16, tag="xs")
                    boff = 0
                    for _ in range(nbun):
                        tn = min(CH, T - t0)
                        gc0 = (int(TOFF[w]) + t0) * 128
                        xe = xep.tile([81, CH, 128], bf16, tag="xe")
                        nc.sync.dma_start(
                            xe[:, :tn, :],
                            xe_d.ap()[:, gc0: gc0 + tn * 128]
                                .rearrange("p (t c) -> p t c", c=128))
                        ps = psp.tile([128, 4, 512], f32, tag="blk")
                        for j in range(tn):
                            nc.tensor.matmul(
                                ps[:, j // 3,
                                   (j % 3) * 132:(j % 3) * 132 + 132],
                                xe[:, j, :], wlg[:], start=True, stop=True)
                        psv = ps[:, :, 0:396] \
                            .rearrange("p b (j c) -> p b j c", c=132)
                        # evacuate xp + u_pre to SBUF bf16 (ACT)
                        nc.scalar.copy(
                            xs[:, boff:boff + CH, :]
                                .rearrange("p (b j) c -> p b j c", b=4), psv)
                        if boff == 0:
                            flush_pend()
                        boff += tn
                        t0 += tn
                        ci += 1
                    bw = boff
                    emit_logits(bt0, bw, xs)
                    is_last = t0 >= T
                    pend[0] = (bt0, bw, xs,
                               emit_close if is_last else None, emit_mult)
            flush_pend()

    nc.compile()
    return nc


def kernel(**inputs):
    from concourse.bass_utils import run_bass_kernel_spmd

    args = {k: np.asarray(v) for k, v in inputs.items()}
    common, per_core = _host_preprocess(
        args["x"], args["edge_index"], args["edge_attr"], args["W_lin"],
        args["w_s"], args["b_s"], args["w_t"], args["b_t"], args["W_edge"],
        args["w_e"], args["b_e"], args["W_res"], args["bias"])

    nc = _build_program(common)

    in_maps = []
    for c in range(NCORES):
        pc = per_core[c]
        in_maps.append({
            "xeT": pc["xeT"], "xtown": pc["xtown"],
            "wlg": common["wlg"], "wrd": common["wrd"],
        })

    res = run_bass_kernel_spmd(nc, in_maps, list(range(NCORES)),
                               trace=bool(os.environ.get("GAT_TRACE")),
                               tmpdir=os.environ.get("GAT_TMPDIR"))
    if os.environ.get("GAT_TRACE"):
        print(f"HW exec time: {res.exec_time_ns} ns")

    out = np.empty((N, HEADS * OUT_F), np.float32)
    for c in range(NCORES):
        out[per_core[c]["perm_owned"]] = res.results[c]["out"][:NODES_PC]
    return out


# revision 20
# speedup vs baseline: 1.1851x; 1.0960x over previous
"""GAT layer (gnn_message_passing) on 8 trn2 NeuronCores.

Strategy (dst-sharded, zero gathers, data-as-weights matmuls):
- Each core owns a contiguous 1/8 slice of target nodes; host buckets edges by
  dst core. Owned nodes are degree-sorted into 128-node windows; node -> SBUF
  partition, its in-edges occupy slot columns t=0..deg-1 (common T_w schedule
  across cores).
- Host lays out, per edge slot, the column [x[src](64) | edge_attr(16) |
  b_total(1)] into xe_slotT [81, SUMT*128] bf16 (pure indexed copy). Padded
  slots get -100 in row 80 so their logits vanish under exp.
- Device, per 128-slot block: ONE matmul with the slot data as the stationary
  operand: out[slot, :] = xe_blk.T @ WLG where WLG [81,132] packs
  [W_lin.T | fold(W_lin,w_s) + C(W_edge,w_e) + bias]. Column 0:128 = xp[src],
  128:132 = a_s[src]+a_e+b_total, already head-major (slots on partitions).
  a_t[dst] + residual come from one per-window matmul of xTown against
  [W_res.T+bias | fold(W_lin,w_t)].
- ACT copies psum->SBUF bf16; DVE adds a_t, leaky-relu (stt), ACT exp writes
  duplicated-pair ev straight into the msg tile; DVE multiplies ev into xp at
  bf16 2x rate (dup-pair broadcast AP keeps operands packed); per-window
  fold tree + axis-swapped tensor_reduce give numerator+denominator in one
  [128,136] result. out = num/denom + residual.
"""
import os
import sys
from contextlib import ExitStack

sys.path.insert(0, "/opt/trn_rl_repo")

import numpy as np
import ml_dtypes

BF16 = ml_dtypes.bfloat16

N, E = 50000, 1600000
IN_F, EDGE_F, HEADS, OUT_F = 64, 16, 4, 32
NEG_SLOPE = 0.2
NCORES = 8
NODES_PC = N // NCORES            # 6250
NW = (NODES_PC + 127) // 128      # 49 windows/core
WNODES = NW * 128                 # 6272
CH = 12                           # slot-cols per chunk (4 psum banks)
PAD_NEG = -100.0                  # row-80 value for invalid slots


def _host_preprocess(x, edge_index, edge_attr, W_lin, w_s, b_s, w_t, b_t,
                     W_edge, w_e, b_e, W_res, bias):
    """Pure index/layout work + weight folding. Returns (common, per_core)."""
    src = edge_index[0].astype(np.int64)
    dst = edge_index[1].astype(np.int64)
    deg = np.bincount(dst, minlength=N)

    # ---- weight folding (weights only; standard operator fusion) ----
    C = (W_edge.reshape(HEADS, OUT_F, EDGE_F) * w_e[None, :, None]).sum(1)  # [4,16]
    D = (W_lin.reshape(HEADS, OUT_F, IN_F) * w_t[None, :, None]).sum(1).T   # [64,4]
    Dws = (W_lin.reshape(HEADS, OUT_F, IN_F) * w_s[None, :, None]).sum(1).T  # [64,4]
    b_total = float(b_s) + float(b_t) + float(b_e)

    wlg = np.zeros((81, 132), np.float32)
    wlg[0:IN_F, 0:128] = W_lin.T.astype(np.float32)
    wlg[0:IN_F, 128:132] = Dws
    wlg[IN_F:80, 128:132] = C.T
    wlg[80, 128:132] = 1.0

    wrd = np.zeros((65, 132), np.float32)
    wrd[0:IN_F, 0:128] = W_res.T.astype(np.float32)
    wrd[IN_F, 0:128] = bias
    wrd[0:IN_F, 128:132] = D

    # ---- per-core schedules (common T_w across cores) ----
    cores = []
    for c in range(NCORES):
        lo = c * NODES_PC
        owned = np.arange(lo, lo + NODES_PC)
        dc = deg[owned]
        order = np.argsort(-dc, kind="stable")
        perm_owned = owned[order]
        dcs = dc[order]
        dcp = np.zeros(WNODES, np.int64)
        dcp[:NODES_PC] = dcs
        tw = dcp.reshape(NW, 128).max(axis=1)
        cores.append(dict(perm_owned=perm_owned, tw=np.maximum(tw, 1)))

    T_w = np.max(np.stack([cc["tw"] for cc in cores]), axis=0)  # [NW]
    for k in range(0, NW - 1, 2):
        T_w[k] = T_w[k + 1] = max(T_w[k], T_w[k + 1])
    TOFF = np.concatenate([[0], np.cumsum(T_w)])
    SUMT = int(TOFF[-1])

    per_core = []
    for c in range(NCORES):
        cc = cores[c]
        perm_owned = cc["perm_owned"]
        pos = np.empty(N, np.int64)
        pos[perm_owned] = np.arange(NODES_PC)

        emask = (dst >= c * NODES_PC) & (dst < (c + 1) * NODES_PC)
        e_ids = np.nonzero(emask)[0]
        d_loc = pos[dst[e_ids]]                      # 0..6249
        eorder = np.argsort(d_loc, kind="stable")
        e_s = e_ids[eorder]
        ds = d_loc[eorder]
        starts = np.searchsorted(ds, np.arange(NODES_PC))
        t_of = np.arange(len(ds)) - starts[ds]
        w_of = ds // 128
        p_of = ds % 128
        cols = (TOFF[w_of] + t_of) * 128 + p_of

        xe = np.zeros((SUMT * 128, 81), np.float32)
        xe[:, 80] = PAD_NEG
        xe[cols, 0:IN_F] = x[src[e_s]]
        xe[cols, IN_F:80] = edge_attr[e_s]
        xe[cols, 80] = b_total
        xeT = np.ascontiguousarray(xe.T).astype(BF16)

        xtown = np.zeros((65, WNODES), np.float32)
        xtown[0:IN_F, 0:NODES_PC] = x[perm_owned].T
        xtown[IN_F] = 1.0

        per_core.append(dict(
            xeT=xeT,
            xtown=xtown.astype(BF16),
            perm_owned=perm_owned,
        ))

    common = dict(T_w=T_w, TOFF=TOFF, SUMT=SUMT,
                  wlg=wlg.astype(BF16), wrd=wrd.astype(BF16))
    return common, per_core


def _build_program(common):
    import concourse.bass as bass
    import concourse.tile as tile
    from concourse import bacc, mybir

    f32 = mybir.dt.float32
    bf16 = mybir.dt.bfloat16
    AL = mybir.AluOpType
    AX = mybir.AxisListType
    T_w, TOFF, SUMT = common["T_w"], common["TOFF"], common["SUMT"]
    TMAX = int(T_w.max())

    nc = bacc.Bacc("TRN2", target_bir_lowering=False, debug=False,
                   num_devices=NCORES, num_swdge_queues=1)

    xe_d = nc.dram_tensor("xeT", [81, SUMT * 128], bf16, kind="ExternalInput")
    xt_d = nc.dram_tensor("xtown", [65, WNODES], bf16, kind="ExternalInput")
    wlg_d = nc.dram_tensor("wlg", [81, 132], bf16, kind="ExternalInput")
    wrd_d = nc.dram_tensor("wrd", [65, 132], bf16, kind="ExternalInput")
    out_d = nc.dram_tensor("out", [WNODES, 128], f32, kind="ExternalOutput")

    with tile.TileContext(nc) as tc, ExitStack() as ctx:
        const = ctx.enter_context(tc.tile_pool(name="const", bufs=1))
        wlg = const.tile([81, 132], bf16)
        nc.sync.dma_start(wlg[:], wlg_d.ap())
        wrd = const.tile([65, 132], bf16)
        nc.sync.dma_start(wrd[:], wrd_d.ap())
        xtown = const.tile([65, WNODES], bf16)
        nc.sync.dma_start(xtown[:], xt_d.ap())

        with tc.tile_pool(name="xep", bufs=8) as xep, \
             tc.tile_pool(name="xsp", bufs=3) as xsp, \
             tc.tile_pool(name="msgp", bufs=2) as msgp, \
             tc.tile_pool(name="up", bufs=3) as up, \
             tc.tile_pool(name="resatp", bufs=3) as resatp, \
             tc.tile_pool(name="foldp", bufs=4) as foldp, \
             tc.tile_pool(name="outp", bufs=4) as outp, \
             tc.tile_pool(name="psp", bufs=2, space="PSUM") as psp:

            pend = [None]      # (wi, t0, bw, xs, close_after, emit_mult)
            def flush_pend():
                if pend[0] is None:
                    return
                wi, t0, bw, xs, close_after, em = pend[0]
                pend[0] = None
                em(wi, t0, bw, xs)
                if close_after is not None:
                    close_after()

            BUN = 4                # chunks per logit/mult bundle
            for pw in range(0, NW, 2):
                P = 2 if pw + 1 < NW else 1
                T = int(T_w[pw])
                # residual + a_t for the pair's 2x128 dst nodes
                ps_r = psp.tile([128, 4, 512], f32, tag="blk")
                for i in range(P):
                    nc.tensor.matmul(ps_r[:, i, 0:132],
                                     xtown[:, (pw + i) * 128:(pw + i + 1) * 128],
                                     wrd[:], start=True, stop=True)
                resat = resatp.tile([128, 2, 132], bf16, tag="resat")
                nc.scalar.copy(resat[:, :P], ps_r[:, 0:P, 0:132])

                msg = msgp.tile([128, 2, TMAX, 128], bf16, tag="msg")
                evd = msgp.tile([128, 2, TMAX, 4, 2], bf16, tag="evd")

                def emit_mult(wi, t0, bw, xs, msg=msg, evd=evd):
                    # msg = ev * xp  (bf16 2x; (t,h) of ev operand merge)
                    evb = evd[:, wi, t0:t0 + bw] \
                        .rearrange("p t h (a two) -> p t h a two", a=1, two=2) \
                        .broadcast_to([128, bw, 4, 16, 2])
                    nc.vector.tensor_tensor(
                        msg[:, wi, t0:t0 + bw, :]
                            .rearrange("p t (h a two) -> p t h a two", h=4, two=2),
                        xs[:, :bw, 0:128]
                            .rearrange("p t (h a two) -> p t h a two", h=4, two=2),
                        evb, op=AL.mult)

                def emit_close(pw=pw, P=P, T=T, msg=msg, evd=evd, resat=resat):
                    # ---- pair fold: numerator tree + denominator reduce ----
                    n = T
                    while n > 2:
                        k = n // 2
                        nc.vector.tensor_tensor(
                            msg[:, :, 0:k, :], msg[:, :, 0:k, :],
                            msg[:, :, n - k:n, :], op=AL.add)
                        n -= k
                    fold = foldp.tile([128, 2, 128], f32, tag="fold")
                    if T >= 2:
                        nc.gpsimd.tensor_tensor(fold[:], msg[:, :, 0, :],
                                                msg[:, :, 1, :], op=AL.add)
                    else:
                        nc.gpsimd.tensor_copy(fold[:], msg[:, :, 0, :])
                    den = foldp.tile([128, 2, 8], f32, tag="den")
                    nc.vector.tensor_reduce(
                        den[:],
                        evd[:, :, 0:T].rearrange("p i t h two -> p i (h two) t"),
                        axis=AX.X, op=AL.add)
                    # ---- close: out = num/denom + residual ----
                    rec = foldp.tile([128, 2, 8], f32, tag="rec")
                    nc.vector.reciprocal(rec[:], den[:])
                    outw = outp.tile([128, 2, 128], f32, tag="outw")
                    for i in range(P):
                        recb = rec[:, i] \
                            .rearrange("p (h a two) -> p h a two", a=1, two=2) \
                            .broadcast_to([128, 4, 16, 2])
                        nc.vector.tensor_tensor(
                            outw[:, i].rearrange("p (h a two) -> p h a two",
                                                 h=4, two=2),
                            fold[:, i].rearrange("p (h a two) -> p h a two",
                                                 h=4, two=2),
                            recb, op=AL.mult)
                    out2 = outp.tile([128, 2, 128], f32, tag="out2")
                    nc.gpsimd.tensor_tensor(out2[:, :P], outw[:, :P],
                                            resat[:, :P, 0:128], op=AL.add)
                    nc.sync.dma_start(
                        out_d.ap()[pw * 128:(pw + P) * 128, :]
                            .rearrange("(i p) f -> p i f", i=P),
                        out2[:, :P])

                def emit_logits(wi, bt0, bw, xs, resat=resat, evd=evd):
                    # u = (a_s + a_e + b_total) + a_t ; leaky-relu; exp
                    u = up.tile([128, BUN * CH, 4], bf16, tag="u")
                    atb = resat[:, wi, 128:132] \
                        .rearrange("p (a h) -> p a h", a=1) \
                        .broadcast_to([128, bw, 4])
                    nc.vector.tensor_tensor(u[:, :bw], xs[:, :bw, 128:132],
                                            atb, op=AL.add)
                    lr = up.tile([128, BUN * CH, 4], bf16, tag="lr")
                    nc.vector.scalar_tensor_tensor(lr[:, :bw], u[:, :bw],
                                                   NEG_SLOPE, u[:, :bw],
                                                   op0=AL.mult, op1=AL.max)
                    lrb = lr[:, :bw, :].rearrange("p t (h a) -> p t h a", a=1) \
                        .broadcast_to([128, bw, 4, 2])
                    nc.scalar.activation(evd[:, wi, bt0:bt0 + bw], lrb,
                                         mybir.ActivationFunctionType.Exp)

                for wi in range(P):
                    w = pw + wi
                    nch = (T + CH - 1) // CH
                    ci = 0
                    t0 = 0
                    while t0 < T:
                        nbun = min(BUN, nch - ci)
                        bt0 = t0
                        xs = xsp.tile([128, BUN * CH, 132], bf16, tag="xs")
                        boff = 0
                        for _ in range(nbun):
                            tn = min(CH, T - t0)
                            gc0 = (int(TOFF[w]) + t0) * 128
                            xe = xep.tile([81, CH, 128], bf16, tag="xe")
                            nc.sync.dma_start(
                                xe[:, :tn, :],
                                xe_d.ap()[:, gc0: gc0 + tn * 128]
                                    .rearrange("p (t c) -> p t c", c=128))
                            ps = psp.tile([128, 4, 512], f32, tag="blk")
                            for j in range(tn):
                                nc.tensor.matmul(
                                    ps[:, j // 3,
                                       (j % 3) * 132:(j % 3) * 132 + 132],
                                    xe[:, j, :], wlg[:], start=True, stop=True)
                            psv = ps[:, :, 0:396] \
                                .rearrange("p b (j c) -> p b j c", c=132)
                            # evacuate xp + u_pre to SBUF bf16 (ACT)
                            nc.scalar.copy(
                                xs[:, boff:boff + CH, :]
                                    .rearrange("p (b j) c -> p b j c", b=4),
                                psv)
                            if boff == 0:
                                flush_pend()
                            boff += tn
                            t0 += tn
                            ci += 1
                        bw = boff
                        emit_logits(wi, bt0, bw, xs)
                        is_last = (wi == P - 1) and t0 >= T
                        pend[0] = (wi, bt0, bw, xs,
                                   emit_close if is_last else None, emit_mult)
            flush_pend()

    nc.compile()
    return nc


def kernel(**inputs):
    from concourse.bass_utils import run_bass_kernel_spmd

    args = {k: np.asarray(v) for k, v in inputs.items()}
    common, per_core = _host_preprocess(
        args["x"], args["edge_index"], args["edge_attr"], args["W_lin"],
        args["w_s"], args["b_s"], args["w_t"], args["b_t"], args["W_edge"],
        args["w_e"], args["b_e"], args["W_res"], args["bias"])

    nc = _build_program(common)

    in_maps = []
    for c in range(NCORES):
        pc = per_core[c]
        in_maps.append({
            "xeT": pc["xeT"], "xtown": pc["xtown"],
            "wlg": common["wlg"], "wrd": common["wrd"],
        })

    res = run_bass_kernel_spmd(nc, in_maps, list(range(NCORES)),
                               trace=bool(os.environ.get("GAT_TRACE")),
                               tmpdir=os.environ.get("GAT_TMPDIR"))
    if os.environ.get("GAT_TRACE"):
        print(f"HW exec time: {res.exec_time_ns} ns")

    out = np.empty((N, HEADS * OUT_F), np.float32)
    for c in range(NCORES):
        out[per_core[c]["perm_owned"]] = res.results[c]["out"][:NODES_PC]
    return out


# revision 21
# speedup vs baseline: 1.1861x; 1.0009x over previous
"""GAT layer (gnn_message_passing) on 8 trn2 NeuronCores.

Strategy (dst-sharded, zero gathers, data-as-weights matmuls):
- Each core owns a contiguous 1/8 slice of target nodes; host buckets edges by
  dst core. Owned nodes are degree-sorted into 128-node windows; node -> SBUF
  partition, its in-edges occupy slot columns t=0..deg-1 (common T_w schedule
  across cores).
- Host lays out, per edge slot, the column [x[src](64) | edge_attr(16) |
  b_total(1)] into xe_slotT [81, SUMT*128] bf16 (pure indexed copy). Padded
  slots get -100 in row 80 so their logits vanish under exp.
- Device, per 128-slot block: ONE matmul with the slot data as the stationary
  operand: out[slot, :] = xe_blk.T @ WLG where WLG [81,132] packs
  [W_lin.T | fold(W_lin,w_s) + C(W_edge,w_e) + bias]. Column 0:128 = xp[src],
  128:132 = a_s[src]+a_e+b_total, already head-major (slots on partitions).
  a_t[dst] + residual come from one per-window matmul of xTown against
  [W_res.T+bias | fold(W_lin,w_t)].
- ACT copies psum->SBUF bf16; DVE adds a_t, leaky-relu (stt), ACT exp writes
  duplicated-pair ev straight into the msg tile; DVE multiplies ev into xp at
  bf16 2x rate (dup-pair broadcast AP keeps operands packed); per-window
  fold tree + axis-swapped tensor_reduce give numerator+denominator in one
  [128,136] result. out = num/denom + residual.
"""
import os
import sys
from contextlib import ExitStack

sys.path.insert(0, "/opt/trn_rl_repo")

import numpy as np
import ml_dtypes

BF16 = ml_dtypes.bfloat16

N, E = 50000, 1600000
IN_F, EDGE_F, HEADS, OUT_F = 64, 16, 4, 32
NEG_SLOPE = 0.2
NCORES = 8
NODES_PC = N // NCORES            # 6250
NW = (NODES_PC + 127) // 128      # 49 windows/core
WNODES = NW * 128                 # 6272
CH = 12                           # slot-cols per chunk (4 psum banks)
PAD_NEG = -100.0                  # row-80 value for invalid slots


def _host_preprocess(x, edge_index, edge_attr, W_lin, w_s, b_s, w_t, b_t,
                     W_edge, w_e, b_e, W_res, bias):
    """Pure index/layout work + weight folding. Returns (common, per_core)."""
    src = edge_index[0].astype(np.int64)
    dst = edge_index[1].astype(np.int64)
    deg = np.bincount(dst, minlength=N)

    # ---- weight folding (weights only; standard operator fusion) ----
    C = (W_edge.reshape(HEADS, OUT_F, EDGE_F) * w_e[None, :, None]).sum(1)  # [4,16]
    D = (W_lin.reshape(HEADS, OUT_F, IN_F) * w_t[None, :, None]).sum(1).T   # [64,4]
    Dws = (W_lin.reshape(HEADS, OUT_F, IN_F) * w_s[None, :, None]).sum(1).T  # [64,4]
    b_total = float(b_s) + float(b_t) + float(b_e)

    wlg = np.zeros((81, 132), np.float32)
    wlg[0:IN_F, 0:128] = W_lin.T.astype(np.float32)
    wlg[0:IN_F, 128:132] = Dws
    wlg[IN_F:80, 128:132] = C.T
    wlg[80, 128:132] = 1.0

    wrd = np.zeros((65, 132), np.float32)
    wrd[0:IN_F, 0:128] = W_res.T.astype(np.float32)
    wrd[IN_F, 0:128] = bias
    wrd[0:IN_F, 128:132] = D

    # ---- per-core schedules (common T_w across cores) ----
    cores = []
    for c in range(NCORES):
        lo = c * NODES_PC
        owned = np.arange(lo, lo + NODES_PC)
        dc = deg[owned]
        order = np.argsort(-dc, kind="stable")
        perm_owned = owned[order]
        dcs = dc[order]
        dcp = np.zeros(WNODES, np.int64)
        dcp[:NODES_PC] = dcs
        tw = dcp.reshape(NW, 128).max(axis=1)
        cores.append(dict(perm_owned=perm_owned, tw=np.maximum(tw, 1)))

    T_w = np.max(np.stack([cc["tw"] for cc in cores]), axis=0)  # [NW]
    for k in range(0, NW - 1, 2):
        T_w[k] = T_w[k + 1] = max(T_w[k], T_w[k + 1])
    TOFF = np.concatenate([[0], np.cumsum(T_w)])
    SUMT = int(TOFF[-1])

    per_core = []
    for c in range(NCORES):
        cc = cores[c]
        perm_owned = cc["perm_owned"]
        pos = np.empty(N, np.int64)
        pos[perm_owned] = np.arange(NODES_PC)

        emask = (dst >= c * NODES_PC) & (dst < (c + 1) * NODES_PC)
        e_ids = np.nonzero(emask)[0]
        d_loc = pos[dst[e_ids]]                      # 0..6249
        eorder = np.argsort(d_loc, kind="stable")
        e_s = e_ids[eorder]
        ds = d_loc[eorder]
        starts = np.searchsorted(ds, np.arange(NODES_PC))
        t_of = np.arange(len(ds)) - starts[ds]
        w_of = ds // 128
        p_of = ds % 128
        cols = (TOFF[w_of] + t_of) * 128 + p_of

        xe = np.zeros((SUMT * 128, 81), np.float32)
        xe[:, 80] = PAD_NEG
        xe[cols, 0:IN_F] = x[src[e_s]]
        xe[cols, IN_F:80] = edge_attr[e_s]
        xe[cols, 80] = b_total
        xeT = np.ascontiguousarray(xe.T).astype(BF16)

        xtown = np.zeros((65, WNODES), np.float32)
        xtown[0:IN_F, 0:NODES_PC] = x[perm_owned].T
        xtown[IN_F] = 1.0

        per_core.append(dict(
            xeT=xeT,
            xtown=xtown.astype(BF16),
            perm_owned=perm_owned,
        ))

    common = dict(T_w=T_w, TOFF=TOFF, SUMT=SUMT,
                  wlg=wlg.astype(BF16), wrd=wrd.astype(BF16))
    return common, per_core


def _build_program(common):
    import concourse.bass as bass
    import concourse.tile as tile
    from concourse import bacc, mybir

    f32 = mybir.dt.float32
    bf16 = mybir.dt.bfloat16
    AL = mybir.AluOpType
    AX = mybir.AxisListType
    T_w, TOFF, SUMT = common["T_w"], common["TOFF"], common["SUMT"]
    TMAX = int(T_w.max())

    nc = bacc.Bacc("TRN2", target_bir_lowering=False, debug=False,
                   num_devices=NCORES, num_swdge_queues=1)

    xe_d = nc.dram_tensor("xeT", [81, SUMT * 128], bf16, kind="ExternalInput")
    xt_d = nc.dram_tensor("xtown", [65, WNODES], bf16, kind="ExternalInput")
    wlg_d = nc.dram_tensor("wlg", [81, 132], bf16, kind="ExternalInput")
    wrd_d = nc.dram_tensor("wrd", [65, 132], bf16, kind="ExternalInput")
    out_d = nc.dram_tensor("out", [WNODES, 128], f32, kind="ExternalOutput")

    with tile.TileContext(nc) as tc, ExitStack() as ctx:
        const = ctx.enter_context(tc.tile_pool(name="const", bufs=1))
        wlg = const.tile([81, 132], bf16)
        nc.sync.dma_start(wlg[:], wlg_d.ap())
        wrd = const.tile([65, 132], bf16)
        nc.sync.dma_start(wrd[:], wrd_d.ap())
        xtown = const.tile([65, WNODES], bf16)
        nc.sync.dma_start(xtown[:], xt_d.ap())

        with tc.tile_pool(name="xep", bufs=8) as xep, \
             tc.tile_pool(name="xsp", bufs=3) as xsp, \
             tc.tile_pool(name="msgp", bufs=2) as msgp, \
             tc.tile_pool(name="up", bufs=3) as up, \
             tc.tile_pool(name="resatp", bufs=3) as resatp, \
             tc.tile_pool(name="foldp", bufs=4) as foldp, \
             tc.tile_pool(name="outp", bufs=4) as outp, \
             tc.tile_pool(name="psp", bufs=2, space="PSUM") as psp:

            pend = [None]      # (wi, t0, bw, xs, close_after, emit_mult)
            def flush_pend():
                if pend[0] is None:
                    return
                wi, t0, bw, xs, close_after, em = pend[0]
                pend[0] = None
                em(wi, t0, bw, xs)
                if close_after is not None:
                    close_after()

            BUN = 4                # chunks per logit/mult bundle
            for pw in range(0, NW, 2):
                P = 2 if pw + 1 < NW else 1
                T = int(T_w[pw])
                # residual + a_t for the pair's 2x128 dst nodes
                ps_r = psp.tile([128, 4, 512], f32, tag="blk")
                for i in range(P):
                    nc.tensor.matmul(ps_r[:, i, 0:132],
                                     xtown[:, (pw + i) * 128:(pw + i + 1) * 128],
                                     wrd[:], start=True, stop=True)
                resat = resatp.tile([128, 2, 132], bf16, tag="resat")
                nc.scalar.copy(resat[:, :P], ps_r[:, 0:P, 0:132])

                msg = msgp.tile([128, 2, TMAX, 128], bf16, tag="msg")
                evd = msgp.tile([128, 2, TMAX, 4, 2], bf16, tag="evd")

                def emit_mult(wi, t0, bw, xs, msg=msg, evd=evd):
                    # msg = ev * xp  (bf16 2x; (t,h) of ev operand merge)
                    evb = evd[:, wi, t0:t0 + bw] \
                        .rearrange("p t h (a two) -> p t h a two", a=1, two=2) \
                        .broadcast_to([128, bw, 4, 16, 2])
                    nc.vector.tensor_tensor(
                        msg[:, wi, t0:t0 + bw, :]
                            .rearrange("p t (h a two) -> p t h a two", h=4, two=2),
                        xs[:, :bw, 0:128]
                            .rearrange("p t (h a two) -> p t h a two", h=4, two=2),
                        evb, op=AL.mult)

                def emit_close(pw=pw, P=P, T=T, msg=msg, evd=evd, resat=resat):
                    # ---- pair fold: numerator tree + denominator reduce ----
                    n = T
                    while n > 2:
                        k = n // 2
                        nc.vector.tensor_tensor(
                            msg[:, :, 0:k, :], msg[:, :, 0:k, :],
                            msg[:, :, n - k:n, :], op=AL.add)
                        n -= k
                    fold = foldp.tile([128, 2, 128], f32, tag="fold")
                    if T >= 2:
                        nc.gpsimd.tensor_tensor(fold[:], msg[:, :, 0, :],
                                                msg[:, :, 1, :], op=AL.add)
                    else:
                        nc.gpsimd.tensor_copy(fold[:], msg[:, :, 0, :])
                    den = foldp.tile([128, 2, 8], f32, tag="den")
                    nc.vector.tensor_reduce(
                        den[:],
                        evd[:, :, 0:T].rearrange("p i t h two -> p i (h two) t"),
                        axis=AX.X, op=AL.add)
                    # ---- close: out = num/denom + residual ----
                    rec = foldp.tile([128, 2, 8], f32, tag="rec")
                    nc.vector.reciprocal(rec[:], den[:])
                    outw = outp.tile([128, 2, 128], f32, tag="outw")
                    for i in range(P):
                        recb = rec[:, i] \
                            .rearrange("p (h a two) -> p h a two", a=1, two=2) \
                            .broadcast_to([128, 4, 16, 2])
                        nc.gpsimd.tensor_tensor(
                            outw[:, i].rearrange("p (h a two) -> p h a two",
                                                 h=4, two=2),
                            fold[:, i].rearrange("p (h a two) -> p h a two",
                                                 h=4, two=2),
                            recb, op=AL.mult)
                    out2 = outp.tile([128, 2, 128], f32, tag="out2")
                    nc.gpsimd.tensor_tensor(out2[:, :P], outw[:, :P],
                                            resat[:, :P, 0:128], op=AL.add)
                    nc.sync.dma_start(
                        out_d.ap()[pw * 128:(pw + P) * 128, :]
                            .rearrange("(i p) f -> p i f", i=P),
                        out2[:, :P])

                def emit_logits(wi, bt0, bw, xs, resat=resat, evd=evd):
                    # u = (a_s + a_e + b_total) + a_t ; leaky-relu; exp
                    u = up.tile([128, BUN * CH, 4], bf16, tag="u")
                    atb = resat[:, wi, 128:132] \
                        .rearrange("p (a h) -> p a h", a=1) \
                        .broadcast_to([128, bw, 4])
                    nc.vector.tensor_tensor(u[:, :bw], xs[:, :bw, 128:132],
                                            atb, op=AL.add)
                    lr = up.tile([128, BUN * CH, 4], bf16, tag="lr")
                    nc.vector.scalar_tensor_tensor(lr[:, :bw], u[:, :bw],
                                                   NEG_SLOPE, u[:, :bw],
                                                   op0=AL.mult, op1=AL.max)
                    lrb = lr[:, :bw, :].rearrange("p t (h a) -> p t h a", a=1) \
                        .broadcast_to([128, bw, 4, 2])
                    nc.scalar.activation(evd[:, wi, bt0:bt0 + bw], lrb,
                                         mybir.ActivationFunctionType.Exp)

                for wi in range(P):
                    w = pw + wi
                    nch = (T + CH - 1) // CH
                    ci = 0
                    t0 = 0
                    while t0 < T:
                        nbun = min(BUN, nch - ci)
                        bt0 = t0
                        xs = xsp.tile([128, BUN * CH, 132], bf16, tag="xs")
                        boff = 0
                        for _ in range(nbun):
                            tn = min(CH, T - t0)
                            gc0 = (int(TOFF[w]) + t0) * 128
                            xe = xep.tile([81, CH, 128], bf16, tag="xe")
                            nc.sync.dma_start(
                                xe[:, :tn, :],
                                xe_d.ap()[:, gc0: gc0 + tn * 128]
                                    .rearrange("p (t c) -> p t c", c=128))
                            ps = psp.tile([128, 4, 512], f32, tag="blk")
                            for j in range(tn):
                                nc.tensor.matmul(
                                    ps[:, j // 3,
                                       (j % 3) * 132:(j % 3) * 132 + 132],
                                    xe[:, j, :], wlg[:], start=True, stop=True)
                            psv = ps[:, :, 0:396] \
                                .rearrange("p b (j c) -> p b j c", c=132)
                            # evacuate xp + u_pre to SBUF bf16 (ACT)
                            nc.scalar.copy(
                                xs[:, boff:boff + CH, :]
                                    .rearrange("p (b j) c -> p b j c", b=4),
                                psv)
                            if boff == 0:
                                flush_pend()
                            boff += tn
                            t0 += tn
                            ci += 1
                        bw = boff
                        emit_logits(wi, bt0, bw, xs)
                        is_last = (wi == P - 1) and t0 >= T
                        pend[0] = (wi, bt0, bw, xs,
                                   emit_close if is_last else None, emit_mult)
            flush_pend()

    nc.compile()
    return nc


def kernel(**inputs):
    from concourse.bass_utils import run_bass_kernel_spmd

    args = {k: np.asarray(v) for k, v in inputs.items()}
    common, per_core = _host_preprocess(
        args["x"], args["edge_index"], args["edge_attr"], args["W_lin"],
        args["w_s"], args["b_s"], args["w_t"], args["b_t"], args["W_edge"],
        args["w_e"], args["b_e"], args["W_res"], args["bias"])

    nc = _build_program(common)

    in_maps = []
    for c in range(NCORES):
        pc = per_core[c]
        in_maps.append({
            "xeT": pc["xeT"], "xtown": pc["xtown"],
            "wlg": common["wlg"], "wrd": common["wrd"],
        })

    res = run_bass_kernel_spmd(nc, in_maps, list(range(NCORES)),
                               trace=bool(os.environ.get("GAT_TRACE")),
                               tmpdir=os.environ.get("GAT_TMPDIR"))
    if os.environ.get("GAT_TRACE"):
        print(f"HW exec time: {res.exec_time_ns} ns")

    out = np.empty((N, HEADS * OUT_F), np.float32)
    for c in range(NCORES):
        out[per_core[c]["perm_owned"]] = res.results[c]["out"][:NODES_PC]
    return out
